# revision 1
# baseline (speedup 1.0000x reference)
# Trainium2 Bass kernel for nn_BottleNeck (sparse local attention bottleneck).
#
# Sharding: data-parallel over batch (B=8 -> 8 cores, one image each).
# BatchNorm batch-statistics are computed as per-core partials and combined
# with three tiny (1-2KB) AllReduce collectives.
#
# On-chip layout: channels on partitions, hw=32*32=1024 on the free dim.
# Channels are PERMUTED so that partitions 0:63 hold the "x-type" attention
# channels (rel depends only on kh) of all 8 groups and 64:127 the "y-type"
# (rel depends only on kw). The permutation is folded into W1/Wq/Wk/Wv/W3 and
# the BN parameters host-side; conv3 un-permutes, so the output is in the
# original channel order.
#
# Attention: for each of the 49 window shifts, l = (k_shift + rel)*q is built
# from shifted AP views of the padded k field (no unfold materialization).
# The (k+rel) add is split across engines for load balance (DVE tensor_scalar
# 4x for kw 2/4/6... see ACT_ADD_KWS / STT_KWS), the *q multiply runs as
# parity-batched bf16 tensor_tensor at DVE 2x (odd shifts read a one-element-
# shifted copy of the field so packed reads stay 4B-aligned), exp runs on the
# scalar engine (1x), and the softmax reductions over the 49 shifts (Z = sum e,
# S = sum e*v) are accumulated on the otherwise-idle TensorEngine as identity
# matmuls into PSUM banks. BN uses exp(-0.5*ln(var+eps)) so the whole kernel
# stays in one ACT table set (no 2.7us table reloads).
#
# Measured (8-core axon trn2, steady-state in-NEFF replication slope):
# ~175-190 us/invocation wall on the slowest core; rel err ~5e-3 vs the fp32
# reference (dominated by bf16 weight quantization).

import os
from contextlib import ExitStack

import numpy as np
import ml_dtypes

import concourse.bass as bass
import concourse.mybir as mybir
import concourse.tile as tile
from concourse import bacc
from concourse.ap import AP
from concourse.bass_utils import run_bass_kernel_spmd

F32 = mybir.dt.float32
BF16 = mybir.dt.bfloat16
AF = mybir.ActivationFunctionType
OP = mybir.AluOpType
AX = mybir.AxisListType

B, C_IN, H, W = 8, 512, 32, 32
PLANES, GROUPS, KS, PAD = 128, 8, 7, 3
D = PLANES // GROUPS
REL = D // 2
HW = H * W
PW = W + 2 * PAD            # 38
PHW = PW * PW               # 1444
EPS = 1e-5
N_CORES = 8
NSAMP = float(B * HW)       # batchnorm sample count over (N,H,W)

# kw plane order inside the per-kh buffers: evens first, then odds, so that
# one AP (kw step 2) covers each parity block contiguously.
KW_ORDER = [0, 2, 4, 6, 1, 3, 5]

# How many of the 49 (k+rel) adds run on ScalarE instead of VectorE (engine
# load balancing; DVE carries the two big multiply passes).
_act_adds = os.environ.get("BASS_ACT_ADDS", "0,1,3")
ACT_ADD_KWS = {int(v) for v in _act_adds.split(",") if v != ""}
_pool_adds = os.environ.get("BASS_POOL_ADDS", "")
POOL_ADD_KWS = {int(v) for v in _pool_adds.split(",") if v != ""}
_CC_MODE = os.environ.get("BASS_CC_MODE", "ar")    # ar=AllReduce, ag=AllGather+local sum
_NO_CC = os.environ.get("BASS_NO_CC") == "1"       # debug: skip collectives
_REPS = int(os.environ.get("BASS_REPS", "1"))      # bench: repeat body in-NEFF
_NO_ATT = os.environ.get("BASS_NO_ATT") == "1"     # debug: skip attention loop


def _sview(flat_ap, off, dims):
    """Hand-built strided view of an SBUF tile ([partition] + dims)."""
    return AP(flat_ap.tensor, off, [list(flat_ap.ap[0])] + [list(d) for d in dims])


def _build_nc():
    nc = bacc.Bacc("TRN2", target_bir_lowering=False, debug=False,
                   num_devices=N_CORES)

    xf_d = nc.dram_tensor("xf", [C_IN, HW], F32, kind="ExternalInput")
    xb_d = nc.dram_tensor("xb", [C_IN, HW], BF16, kind="ExternalInput")
    w1t_d = nc.dram_tensor("w1t", [C_IN, PLANES], BF16, kind="ExternalInput")
    mqkv_d = nc.dram_tensor("mqkv", [3, PLANES, PLANES], BF16, kind="ExternalInput")
    w3t_d = nc.dram_tensor("w3t", [PLANES, 4 * PLANES], BF16, kind="ExternalInput")
    bkv_d = nc.dram_tensor("bkv", [PLANES, 2], F32, kind="ExternalInput")
    relc_d = nc.dram_tensor("relc", [PLANES, KS * KS], F32, kind="ExternalInput")
    gb12_d = nc.dram_tensor("gb12", [PLANES, 4], F32, kind="ExternalInput")
    gb3_d = nc.dram_tensor("gb3", [PLANES, 8], F32, kind="ExternalInput")
    id_d = nc.dram_tensor("id128", [PLANES, PLANES], BF16, kind="ExternalInput")
    out_d = nc.dram_tensor("out", [C_IN, HW], F32, kind="ExternalOutput")

    dbg = os.environ.get("BASS_KDBG") == "1"
    if dbg:
        dbg_d = {n: nc.dram_tensor(f"dbg_{n}", shp, F32, kind="ExternalOutput")
                 for n, shp in [("o1", [128, HW]), ("x1p", [128, PHW]),
                                ("q", [128, HW]), ("kf", [128, PHW]),
                                ("eb0", [128, 7 * HW]), ("z", [128, HW]),
                                ("s", [128, HW]), ("att", [128, HW]),
                                ("o3", [128, 4 * HW])]}

    with tile.TileContext(nc) as tc, ExitStack() as ctx:
        const = ctx.enter_context(tc.tile_pool(name="const", bufs=1))
        sb = ctx.enter_context(tc.tile_pool(name="sb", bufs=1))
        work = ctx.enter_context(tc.tile_pool(name="work", bufs=2))
        psum = ctx.enter_context(tc.tile_pool(name="psum", bufs=1, space="PSUM"))
        dram = ctx.enter_context(tc.tile_pool(name="dram", bufs=1, space="DRAM"))

        # ---------------- constants / weights ----------------
        id_sb = const.tile([128, 128], BF16)
        nc.sync.dma_start(id_sb[:], id_d[:])
        w1t_sb = const.tile([128, 4, 128], BF16)
        for k in range(4):
            nc.sync.dma_start(w1t_sb[:, k, :], w1t_d[k * 128:(k + 1) * 128, :])
        mqkv_sb = const.tile([128, 3, 128], BF16)
        for i in range(3):
            nc.sync.dma_start(mqkv_sb[:, i, :], mqkv_d[i])
        w3t_sb = const.tile([128, 512], BF16)
        nc.sync.dma_start(w3t_sb[:], w3t_d[:])
        bkv_sb = const.tile([128, 2], F32)
        nc.sync.dma_start(bkv_sb[:], bkv_d[:])
        relc_sb = const.tile([128, 49], F32)
        nc.sync.dma_start(relc_sb[:], relc_d[:])
        gb12_sb = const.tile([128, 4], F32)
        nc.sync.dma_start(gb12_sb[:], gb12_d[:])
        gb3_sb = const.tile([128, 8], F32)
        nc.sync.dma_start(gb3_sb[:], gb3_d[:])

        zcol = const.tile([128, 1], F32)
        nc.gpsimd.memset(zcol[:], 0.0)
        expwarm = const.tile([128, 1], F32)
        nc.scalar.activation(expwarm[:], zcol[:], AF.Exp, bias=zcol[:])
        epscol = const.tile([128, 1], F32)
        nc.gpsimd.memset(epscol[:], EPS)

        for _rep in range(_REPS):
            xb_sb = sb.tile([128, 4, HW], BF16)
            for k in range(4):
                for hh in range(2):
                    nc.sync.dma_start(
                        xb_sb[:, k, hh * 512:(hh + 1) * 512],
                        xb_d[k * 128:(k + 1) * 128, hh * 512:(hh + 1) * 512])

            # ---------------- helpers ----------------
            def dump(name, ap):
                if not dbg:
                    return
                n = ap.free_size()
                scr = work.tile([128, 7 * HW], F32, tag="dbgscr", bufs=1,
                                name=f"dbgscr_{name}")[:, 0:n]
                nc.vector.tensor_copy(scr[:], ap)
                nc.sync.dma_start(dbg_d[name][:], scr[:])

            def allreduce(src_ap, ncols, name):
                dst = sb.tile([128, ncols], F32, name=f"cc_{name}_res")
                if _NO_CC:
                    nc.vector.tensor_scalar_mul(dst[:], src_ap, float(N_CORES))
                    return dst
                cin = dram.tile([128, ncols], F32, name=f"cc_{name}_in")
                nc.sync.dma_start(cin[:], src_ap)
                if _CC_MODE == "ag":
                    cout = dram.tile([N_CORES * 128, ncols], F32,
                                     name=f"cc_{name}_out")
                    nc.gpsimd.collective_compute(
                        "AllGather", OP.bypass,
                        replica_groups=[list(range(N_CORES))],
                        ins=[cin[:].opt()], outs=[cout[:].opt()],
                    )
                    gat = sb.tile([128, ncols, N_CORES], F32, name=f"cc_{name}_gat")
                    nc.sync.dma_start(
                        gat[:], cout[:].rearrange("(j p) c -> p c j", p=128))
                    nc.vector.tensor_reduce(dst[:], gat[:], AX.X, OP.add)
                else:
                    cout = dram.tile([128, ncols], F32, name=f"cc_{name}_out")
                    nc.gpsimd.collective_compute(
                        "AllReduce", OP.add,
                        replica_groups=[list(range(N_CORES))],
                        ins=[cin[:].opt()], outs=[cout[:].opt()],
                    )
                    nc.sync.dma_start(dst[:], cout[:])
                return dst

            def bn_params(gsum, gsumsq, gamma, beta, name):
                """a = gamma*rsqrt(var+eps), b = beta - a*mean  (columns [128,1])."""
                mean = sb.tile([128, 1], F32, name=f"bn_{name}_mean")
                msq = sb.tile([128, 1], F32, name=f"bn_{name}_msq")
                nc.scalar.activation(mean[:], gsum, AF.Copy, scale=1.0 / NSAMP)
                nc.scalar.activation(msq[:], gsumsq, AF.Copy, scale=1.0 / NSAMP)
                var = sb.tile([128, 1], F32, name=f"bn_{name}_var")
                nc.vector.tensor_tensor(var[:], mean[:], mean[:], OP.mult)
                nc.vector.tensor_tensor(var[:], msq[:], var[:], OP.subtract)
                lg = sb.tile([128, 1], F32, name=f"bn_{name}_lg")
                nc.scalar.activation(lg[:], var[:], AF.Ln, bias=epscol[:])
                rsd = sb.tile([128, 1], F32, name=f"bn_{name}_rsd")
                nc.scalar.activation(rsd[:], lg[:], AF.Exp, bias=zcol[:], scale=-0.5)
                a = sb.tile([128, 1], F32, name=f"bn_{name}_a")
                b = sb.tile([128, 1], F32, name=f"bn_{name}_b")
                nc.vector.tensor_tensor(a[:], gamma, rsd[:], OP.mult)
                nc.vector.tensor_tensor(b[:], a[:], mean[:], OP.mult)
                nc.vector.tensor_tensor(b[:], beta, b[:], OP.subtract)
                return a, b

            # ---------------- conv1 (512 -> 128) ----------------
            o1_ps = psum.tile([128, HW], F32, tag="A")
            for n in range(2):
                for k in range(4):
                    nc.tensor.matmul(
                        o1_ps[:, n * 512:(n + 1) * 512],
                        w1t_sb[:, k, :],
                        xb_sb[:, k, n * 512:(n + 1) * 512],
                        start=(k == 0), stop=(k == 3),
                    )

            s1 = sb.tile([128, 2], F32)
            nc.vector.tensor_reduce(s1[:, 0:1], o1_ps[:], AX.X, OP.add)
            sq1 = work.tile([128, HW], BF16, tag="sq", bufs=1)
            nc.scalar.activation(sq1[:], o1_ps[:], AF.Square, bias=zcol[:],
                                 accum_out=s1[:, 1:2])
            dump("o1", o1_ps[:])
            g1 = allreduce(s1[:], 2, "bn1")
            a1, b1 = bn_params(g1[:, 0:1], g1[:, 1:2],
                               gb12_sb[:, 0:1], gb12_sb[:, 1:2], "1")

            # x1 = relu(a1*o1 + b1), written into the zero-padded 38x38 field
            x1p = sb.tile([128, PW, PW], BF16)
            nc.gpsimd.memset(x1p[:], 0.0)
            nc.scalar.activation(
                x1p[:, PAD:PAD + H, PAD:PAD + W],
                o1_ps[:].rearrange("p (y x) -> p y x", y=H),
                AF.Relu, bias=b1[:], scale=a1[:],
            )

            dump("x1p", x1p[:].rearrange("p y x -> p (y x)"))
            # ---------------- q/k/v grouped 1x1 convs ----------------

            col_splits = [(0, 512), (512, 1024), (1024, PHW)]

            def kv_conv(widx, bias_col, name, ps_tag):
                ps = psum.tile([128, PHW], F32, tag=ps_tag, name=f"{name}_ps")
                x1p_flat = x1p[:].rearrange("p y x -> p (y x)")
                for (c0, c1) in col_splits:
                    nc.tensor.matmul(
                        ps[:, c0:c1],
                        mqkv_sb[:, widx, :],
                        x1p_flat[:, c0:c1],
                        start=True, stop=True,
                    )
                fld = sb.tile([128, PW, PW], BF16, name=f"{name}_fld")
                nc.scalar.activation(
                    fld[:], ps[:].rearrange("p (y x) -> p y x", y=PW),
                    AF.Identity, bias=bias_col,
                )
                odd = sb.tile([128, PHW - 1], BF16, name=f"{name}_odd")
                nc.vector.tensor_copy(
                    odd[:], fld[:].rearrange("p y x -> p (y x)")[:, 1:PHW])
                return fld, odd

            q_ps = psum.tile([128, HW], F32, tag="A")
            for n in range(2):
                nc.tensor.matmul(
                    q_ps[:, n * 512:(n + 1) * 512],
                    mqkv_sb[:, 0, :],
                    x1p[:, PAD + n * 16:PAD + (n + 1) * 16, PAD:PAD + W],
                    start=True, stop=True,
                )
            q_bf = sb.tile([128, HW], BF16)
            nc.scalar.activation(q_bf[:], q_ps[:], AF.Copy)
            dump("q", q_bf[:])
            k_fld, k_odd = kv_conv(1, bkv_sb[:, 0:1], "k", "B")
            dump("kf", k_fld[:].rearrange("p y x -> p (y x)"))
            v_fld, v_odd = kv_conv(2, bkv_sb[:, 1:2], "v", "C")

            # ---------------- attention over 49 shifts ----------------
            z_ps = psum.tile([128, HW], F32, tag="A")
            s_ps = psum.tile([128, HW], F32, tag="B")

            STT_KWS = {6, 5}     # fused (k+rel)*q planes (last of each parity block)
            for kh in range([0, KS][not _NO_ATT]):
                mb = work.tile([128, KS, HW], BF16, tag="mb")
                q3 = q_bf[:].rearrange("p (y x) -> p y x", y=H)
                # (k_shift + rel) per plane; rel is a per-partition scalar
                for pos, kw in enumerate(KW_ORDER):
                    kap = kh * KS + kw
                    if kw % 2 == 0:
                        view = k_fld[:, kh:kh + H, kw:kw + W]
                    else:
                        view = _sview(k_odd[:], kh * PW + kw - 1,
                                      [(PW, H), (1, W)])
                    dst = mb[:, pos, :].rearrange("p (y x) -> p y x", y=H)
                    if kw in STT_KWS:
                        nc.vector.scalar_tensor_tensor(
                            dst, view, relc_sb[:, kap:kap + 1], q3,
                            OP.add, OP.mult)
                    elif kw in ACT_ADD_KWS:
                        nc.scalar.activation(dst, view, AF.Identity,
                                             bias=relc_sb[:, kap:kap + 1])
                    elif kw in POOL_ADD_KWS:
                        nc.gpsimd.tensor_scalar_add(dst, view,
                                                    relc_sb[:, kap:kap + 1])
                    else:
                        nc.vector.tensor_scalar_add(dst, view,
                                                    relc_sb[:, kap:kap + 1])
                # l = m * q  (parity-batched, bf16 2x, in place over mb;
                # planes 3 (kw6) and 6 (kw5) already hold l via the fused op)
                nc.vector.tensor_tensor(
                    mb[:, 0:3, :], mb[:, 0:3, :],
                    q_bf[:].rearrange("p (a h) -> p a h", a=1).to_broadcast([128, 3, HW]),
                    OP.mult)
                nc.vector.tensor_tensor(
                    mb[:, 4:6, :], mb[:, 4:6, :],
                    q_bf[:].rearrange("p (a h) -> p a h", a=1).to_broadcast([128, 2, HW]),
                    OP.mult)
                # e = exp(l)
                eb = work.tile([128, KS, HW], BF16, tag="eb", bufs=3)
                nc.scalar.activation(eb[:], mb[:], AF.Exp, bias=zcol[:])
                # Z += sum_planes(e) on the TensorEngine
                for pl in range(KS):
                    for hh in range(2):
                        nc.tensor.matmul(
                            z_ps[:, hh * 512:(hh + 1) * 512],
                            id_sb[:], eb[:, pl, hh * 512:(hh + 1) * 512],
                            start=(kh == 0 and pl == 0), stop=(kh == KS - 1 and pl == KS - 1),
                            skip_group_check=True,
                        )
                if kh == 0:
                    dump("eb0", eb[:].rearrange("p k h -> p (k h)"))
                # ev = e * v_shift (parity-batched)
                evb = work.tile([128, KS, HW], BF16, tag="evb")
                v_flat = v_fld[:].rearrange("p y x -> p (y x)")
                ev_even = _sview(v_flat, kh * PW, [(2, 4), (PW, H), (1, W)])
                ev_odd = _sview(v_odd[:], kh * PW, [(2, 3), (PW, H), (1, W)])
                nc.vector.tensor_tensor(
                    evb[:, 0:4, :].rearrange("p k (y x) -> p k y x", y=H),
                    eb[:, 0:4, :].rearrange("p k (y x) -> p k y x", y=H),
                    ev_even, OP.mult)
                nc.vector.tensor_tensor(
                    evb[:, 4:7, :].rearrange("p k (y x) -> p k y x", y=H),
                    eb[:, 4:7, :].rearrange("p k (y x) -> p k y x", y=H),
                    ev_odd, OP.mult)
                # S += sum_planes(ev)
                for pl in range(KS):
                    for hh in range(2):
                        nc.tensor.matmul(
                            s_ps[:, hh * 512:(hh + 1) * 512],
                            id_sb[:], evb[:, pl, hh * 512:(hh + 1) * 512],
                            start=(kh == 0 and pl == 0), stop=(kh == KS - 1 and pl == KS - 1),
                            skip_group_check=True,
                        )

            # att = S / Z, then BN2 + relu
            dump("z", z_ps[:])
            dump("s", s_ps[:])
            rz = sb.tile([128, HW], F32)
            nc.vector.reciprocal_approx_fast(rz[:], z_ps[:])
            att = sb.tile([128, HW], F32)
            nc.vector.tensor_tensor(att[:], s_ps[:], rz[:], OP.mult)

            dump("att", att[:])
            s2 = sb.tile([128, 2], F32)
            nc.vector.tensor_reduce(s2[:, 0:1], att[:], AX.X, OP.add)
            sq2 = work.tile([128, HW], BF16, tag="sq", bufs=1)
            nc.scalar.activation(sq2[:], att[:], AF.Square, bias=zcol[:],
                                 accum_out=s2[:, 1:2])
            g2 = allreduce(s2[:], 2, "bn2")
            a2, b2 = bn_params(g2[:, 0:1], g2[:, 1:2],
                               gb12_sb[:, 2:3], gb12_sb[:, 3:4], "2")
            x2 = sb.tile([128, HW], BF16)
            nc.scalar.activation(x2[:], att[:], AF.Relu, bias=b2[:], scale=a2[:])

            # ---------------- conv3 (128 -> 512) + BN3 + residual ----------------
            xf_sb = sb.tile([128, 4, HW], F32)
            for k in range(4):
                nc.sync.dma_start(xf_sb[:, k, :], xf_d[k * 128:(k + 1) * 128, :])
            o3_sb = sb.tile([128, 4, HW], F32)
            s3 = sb.tile([128, 8], F32)
            for j in range(4):
                o3_ps = psum.tile([128, HW], F32, tag=["C", "A"][j % 2], name=f"o3_ps_{j}")
                for n in range(2):
                    nc.tensor.matmul(
                        o3_ps[:, n * 512:(n + 1) * 512],
                        w3t_sb[:, j * 128:(j + 1) * 128],
                        x2[:, n * 512:(n + 1) * 512],
                        start=True, stop=True,
                    )
                sq3 = work.tile([128, HW], BF16, tag="sq", bufs=1, name=f"sq3_{j}")
                nc.scalar.activation(sq3[:], o3_ps[:], AF.Square, bias=zcol[:],
                                     accum_out=s3[:, 2 * j + 1:2 * j + 2])
                nc.scalar.activation(o3_sb[:, j, :], o3_ps[:], AF.Copy,
                                     accum_out=s3[:, 2 * j:2 * j + 1])

            dump("o3", o3_sb[:].rearrange("p j h -> p (j h)"))
            g3 = allreduce(s3[:], 8, "bn3")
            # batched BN3 params for all 4 chunks: [128, 4] columns
            sc3 = sb.tile([128, 8], F32)
            nc.scalar.activation(sc3[:], g3[:], AF.Copy, scale=1.0 / NSAMP)
            mean3 = sc3[:, 0:8:2]
            msq3 = sc3[:, 1:8:2]
            m23 = sb.tile([128, 4], F32)
            nc.vector.tensor_tensor(m23[:], mean3, mean3, OP.mult)
            var3 = sb.tile([128, 4], F32)
            nc.vector.tensor_tensor(var3[:], msq3, m23[:], OP.subtract)
            lg3 = sb.tile([128, 4], F32)
            nc.scalar.activation(lg3[:], var3[:], AF.Ln, bias=epscol[:])
            rsd3 = sb.tile([128, 4], F32)
            nc.scalar.activation(rsd3[:], lg3[:], AF.Exp, bias=zcol[:], scale=-0.5)
            a3 = sb.tile([128, 4], F32)
            b3 = sb.tile([128, 4], F32)
            nc.vector.tensor_tensor(a3[:], gb3_sb[:, 0:8:2], rsd3[:], OP.mult)
            nc.vector.tensor_tensor(b3[:], a3[:], mean3, OP.mult)
            nc.vector.tensor_tensor(b3[:], gb3_sb[:, 1:8:2], b3[:], OP.subtract)
            for j in range(4):
                t3 = work.tile([128, HW], F32, tag="t3", name=f"t3_{j}")
                nc.vector.scalar_tensor_tensor(t3[:], o3_sb[:, j, :], a3[:, j:j + 1],
                                               xf_sb[:, j, :], OP.mult, OP.add)
                ot = work.tile([128, HW], F32, tag="ot", name=f"ot_{j}")
                nc.scalar.activation(ot[:], t3[:], AF.Relu, bias=b3[:, j:j + 1])
                nc.sync.dma_start(out_d[j * 128:(j + 1) * 128, :], ot[:])

    nc.compile()
    return nc


_NC = None


def _get_nc():
    global _NC
    if _NC is None:
        _NC = _build_nc()
    return _NC


def _prep_inputs(x, W1, g1, b1, Wq, Wk, bk, Wv, bv, rel_x, rel_y, g2, b2, W3, g3, b3):
    f32 = np.float32
    bf = ml_dtypes.bfloat16

    # channel permutation: new partition -> old channel within the 128 planes
    perm = np.zeros(PLANES, dtype=np.int64)
    for g in range(GROUPS):
        for d in range(D):
            p = g * REL + d if d < REL else 64 + g * REL + (d - REL)
            perm[p] = g * D + d

    W1p = np.ascontiguousarray(W1[perm, :])                     # [128, 512]
    w1t = np.ascontiguousarray(W1p.T).astype(bf)                # [512, 128]

    def block_mat(Wg):
        M = np.zeros((PLANES, PLANES), dtype=f32)
        for po in range(PLANES):
            g = (po % 64) // REL
            o = perm[po] - g * D
            for pi_d in range(D):
                pi = g * REL + pi_d if pi_d < REL else 64 + g * REL + (pi_d - REL)
                M[po, pi] = Wg[g, o, pi_d]
        return M

    mqkv = np.stack([np.ascontiguousarray(block_mat(Wg).T)
                     for Wg in (Wq, Wk, Wv)]).astype(bf)        # [3,128,128] (lhsT)

    bkv = np.stack([bk.reshape(-1)[perm], bv.reshape(-1)[perm]], axis=1).astype(f32)

    # rel columns [128, 49]
    relc = np.zeros((PLANES, KS * KS), dtype=f32)
    for p in range(PLANES):
        g = (p % 64) // REL
        dd = perm[p] - g * D
        for kap in range(KS * KS):
            kh, kw = divmod(kap, KS)
            relc[p, kap] = rel_x[dd, kh, 0] if dd < REL else rel_y[dd - REL, 0, kw]

    gb12 = np.stack([g1[perm], b1[perm], g2[perm], b2[perm]], axis=1).astype(f32)

    W3p = np.ascontiguousarray(W3[:, perm])                     # [512, 128]
    w3t = np.ascontiguousarray(W3p.T).astype(bf)                # [128, 512]

    gb3 = np.zeros((PLANES, 8), dtype=f32)
    for j in range(4):
        gb3[:, 2 * j] = g3[j * 128:(j + 1) * 128]
        gb3[:, 2 * j + 1] = b3[j * 128:(j + 1) * 128]

    id128 = np.eye(PLANES, dtype=f32).astype(bf)

    shared = dict(w1t=w1t, mqkv=mqkv, w3t=w3t, bkv=bkv, relc=relc,
                  gb12=gb12, gb3=gb3, id128=id128)
    in_maps = []
    for c in range(N_CORES):
        xi = np.ascontiguousarray(x[c].reshape(C_IN, HW)).astype(f32)
        m = dict(shared)
        m["xf"] = xi
        m["xb"] = xi.astype(bf)
        in_maps.append(m)
    return in_maps


def _run(inputs, **kw):
    nc = _get_nc()
    in_maps = _prep_inputs(**inputs)
    res = run_bass_kernel_spmd(nc, in_maps, core_ids=list(range(N_CORES)), **kw)
    out = np.stack([res.results[c]["out"].reshape(C_IN, H, W)
                    for c in range(N_CORES)]).astype(np.float32)
    return out, res


def kernel(**inputs):
    out, _ = _run(inputs)
    return out

